# revision 21
# baseline (speedup 1.0000x reference)
"""Trainium2 Bass kernel for the sparse_attention nn.Module problem.

Strategy: data-parallel over the MSA-row dim S (S=128 -> 16 rows per core,
8 cores). All projection weights + pair bias replicated; mask bias and
activations sharded with S. No collectives.

Per-core dataflow (all layouts chosen so no on-device input transposes are
needed; host pre-transposes x to [s, c, q]):
  qT/kT = W @ x^T            (fp32r matmuls, PSUM fp32, DVE evict to SBUF)
  v     = kv_x @ Wv^T        (natural [k, t] layout, evicted to bf16 with a
                              ones column per head for the softmax sum)
  g     = q_x @ Wg^T + bg    (bg added via a rank-1 K=1 matmul; sigmoid done
                              as 0.5*(1+tanh(x/2)) with the 0.5 folded into Wo)
  sT_h  = kT_h^T @ qT_h      (scores transposed: [k, q], per head)
  expS  = exp(sT + mask)     (ACT, mask is per-partition bias; no max-sub --
                              |logits| <= ~70 so fp32/bf16 exp is safe)
  A     = expS * exp(pairT)  (exp(bias_pair) precomputed on host, bf16)
  o,Z   = A^T @ [v | 1]      (AV matmul in natural layout, N=33 per head;
                              col 32 accumulates Z = sum_k A)
  og    = (gs+1) * (o * (1/Z))
  ogT   = PE transpose(og)
  out   = ogT^T @ (0.5*Wo)^T + bo
"""

import os
import numpy as np
import ml_dtypes

def _mmdt():
    return (ml_dtypes.bfloat16 if os.environ.get('KDTYPE', 'bf16') == 'bf16'
            else np.float16)

B, S, Q, C = 1, 128, 256, 256
H, DH = 8, 32
TOT = H * DH
N_CORES = 8
S_LOC = S // N_CORES  # 16

_CACHE = {}


def _build_program(s_loc):
    import concourse.bacc as bacc
    import concourse.mybir as mybir
    from concourse import tile

    dt = mybir.dt
    f32, bf16 = dt.float32, dt.bfloat16
    f16 = bf16 if os.environ.get('KDTYPE', 'bf16') == 'bf16' else dt.float16
    AF = mybir.ActivationFunctionType
    ALU = mybir.AluOpType

    nc = bacc.Bacc("TRN2", target_bir_lowering=False, debug=False,
                   num_devices=N_CORES)

    xq_d = nc.dram_tensor("xq", [s_loc, C, Q], f16, kind="ExternalInput").ap()
    xkv_d = nc.dram_tensor("xkv", [s_loc, C, Q], f16, kind="ExternalInput").ap()
    mask_d = nc.dram_tensor("maskt", [128, 2 * s_loc], f32, kind="ExternalInput").ap()
    expb_d = nc.dram_tensor("expb", [128, 2 * H * Q], bf16, kind="ExternalInput").ap()
    wq_d = nc.dram_tensor("wq", [128, 512], f16, kind="ExternalInput").ap()
    wk_d = nc.dram_tensor("wk", [128, 512], f16, kind="ExternalInput").ap()
    wv_d = nc.dram_tensor("wv", [128, 512], f16, kind="ExternalInput").ap()
    wg_d = nc.dram_tensor("wg", [128, 512], f16, kind="ExternalInput").ap()
    wo_d = nc.dram_tensor("wo", [128, 512], f16, kind="ExternalInput").ap()
    bg_d = nc.dram_tensor("bg", [1, 256], f16, kind="ExternalInput").ap()
    bo_d = nc.dram_tensor("bo", [128, 256], f32, kind="ExternalInput").ap()
    id_d = nc.dram_tensor("ident", [128, 128], f16, kind="ExternalInput").ap()
    ones_d = nc.dram_tensor("ones", [1, 128], f16, kind="ExternalInput").ap()
    out_d = nc.dram_tensor("out", [s_loc, Q, C], f32, kind="ExternalOutput").ap()

    def r(ap):
        return ap

    with tile.TileContext(nc) as tc:
        with (
            tc.tile_pool(name="const", bufs=1) as cp,
            tc.tile_pool(name="work", bufs=2) as wp,
            tc.tile_pool(name="ps_small", bufs=2, space="PSUM") as pss,
            tc.tile_pool(name="ps_sc", bufs=2, space="PSUM") as psc,
            tc.tile_pool(name="ps_o", bufs=2, space="PSUM") as pso,
        ):
            # ---- resident constants ----
            wq_t = cp.tile([128, 512], f16, tag="wq")
            wk_t = cp.tile([128, 512], f16, tag="wk")
            wv_t = cp.tile([128, 512], f16, tag="wv")
            wg_t = cp.tile([128, 512], f16, tag="wg")
            wo_t = cp.tile([128, 512], f16, tag="wo")
            expb_t = cp.tile([128, 2 * H * Q], bf16, tag="expb")
            mask_t = cp.tile([128, 2 * s_loc], f32, tag="mask")
            bg_t = cp.tile([1, 256], f16, tag="bg")
            bo_t = cp.tile([128, 256], f32, tag="bo")
            id_t = cp.tile([128, 128], f16, tag="ident")
            ones_t = cp.tile([1, 128], f16, tag="ones")

            nc.sync.dma_start(wq_t[:, :], wq_d[:, :])
            nc.sync.dma_start(wk_t[:, :], wk_d[:, :])
            nc.sync.dma_start(wv_t[:, :], wv_d[:, :])
            nc.sync.dma_start(wg_t[:, :], wg_d[:, :])
            nc.sync.dma_start(wo_t[:, :], wo_d[:, :])
            nc.sync.dma_start(expb_t[:, :], expb_d[:, :])
            nc.sync.dma_start(mask_t[:, :], mask_d[:, :])
            nc.sync.dma_start(bg_t[:, :], bg_d[:, :])
            nc.sync.dma_start(bo_t[:, :], bo_d[:, :])
            nc.sync.dma_start(id_t[:, :], id_d[:, :])
            nc.sync.dma_start(ones_t[:, :], ones_d[:, :])

            for s in range(s_loc):
                # ---- load x^T shards ----
                xq = wp.tile([128, 512], f16, tag="xq")
                xkv = wp.tile([128, 512], f16, tag="xkv")
                nc.sync.dma_start(
                    xq[:, :].rearrange("p (cc q) -> p cc q", cc=2),
                    xq_d[s].rearrange("(cc p) q -> p cc q", p=128))
                nc.sync.dma_start(
                    xkv[:, :].rearrange("p (cc q) -> p cc q", cc=2),
                    xkv_d[s].rearrange("(cc p) q -> p cc q", p=128))

                # ---- projections (fp32r) ----
                # qT[t, q] += WqT[c, t]^T @ xqT[c, q]
                qt_ps = pss.tile([128, 512], f32, tag="pss")
                for tcc in range(2):
                    for cc in range(2):
                        nc.tensor.matmul(
                            qt_ps[:, tcc * 256:(tcc + 1) * 256],
                            r(wq_t[:, cc * 256 + tcc * 128: cc * 256 + tcc * 128 + 128]),
                            r(xq[:, cc * 256:(cc + 1) * 256]),
                            start=(cc == 0), stop=(cc == 1))
                qt = wp.tile([128, 512], f16, tag="qt")
                ev_q = nc.vector.tensor_copy(qt[:, :], qt_ps[:, :])

                kt_ps = pss.tile([128, 512], f32, tag="pss")
                for tcc in range(2):
                    for cc in range(2):
                        nc.tensor.matmul(
                            kt_ps[:, tcc * 256:(tcc + 1) * 256],
                            r(wk_t[:, cc * 256 + tcc * 128: cc * 256 + tcc * 128 + 128]),
                            r(xkv[:, cc * 256:(cc + 1) * 256]),
                            start=(cc == 0), stop=(cc == 1))
                kt = wp.tile([128, 512], f16, tag="kt")
                ev_k = nc.vector.tensor_copy(kt[:, :], kt_ps[:, :])

                # v natural [k, t]
                v_ps = pss.tile([128, 512], f32, tag="pss")
                for kc in range(2):
                    for cc in range(2):
                        nc.tensor.matmul(
                            v_ps[:, kc * 256:(kc + 1) * 256],
                            r(xkv[:, cc * 256 + kc * 128: cc * 256 + kc * 128 + 128]),
                            r(wv_t[:, cc * 256:(cc + 1) * 256]),
                            start=(cc == 0), stop=(cc == 1))
                # v_aug bf16 [k, (kc, h, 33)]; col 32 of each head = 1.0
                v_sb = wp.tile([128, 528], bf16, tag="v")
                v4 = v_sb.rearrange("p (kc h e) -> p kc h e", kc=2, h=8)
                nc.gpsimd.memset(v4[:, :, :, 32], 1.0)
                for kc in range(2):
                    nc.vector.tensor_copy(
                        v4[:, kc, :, 0:32],
                        v_ps[:, kc * 256:(kc + 1) * 256].rearrange(
                            "p (h d) -> p h d", h=8))

                # g natural [q, t] with bg via rank-1 matmul
                g_ps = pss.tile([128, 512], f32, tag="pss")
                for qc in range(2):
                    for cc in range(2):
                        nc.tensor.matmul(
                            g_ps[:, qc * 256:(qc + 1) * 256],
                            r(xq[:, cc * 256 + qc * 128: cc * 256 + qc * 128 + 128]),
                            r(wg_t[:, cc * 256:(cc + 1) * 256]),
                            start=(cc == 0), stop=False)
                    nc.tensor.matmul(
                        g_ps[:, qc * 256:(qc + 1) * 256],
                        r(ones_t[:, :]), r(bg_t[:, :]),
                        start=False, stop=True)
                # gs = tanh(g/2); sigmoid = 0.5*(gs+1), 0.5 folded into Wo
                gs = wp.tile([128, 512], f32, tag="gs")
                nc.scalar.activation(gs[:, :], g_ps[:, :], AF.Tanh, scale=0.5)

                # ---- attention ----
                # Mixing PE tile positions crashes this runtime, so every
                # matmul must sit at partition base 0: DMA-remap qt/kt from
                # [(hh,d), (tc,q)] to head-flat [d, (tc,hh,q)].
                qt2 = wp.tile([32, 2048], f16, tag="qt2")
                kt2 = wp.tile([32, 2048], f16, tag="kt2")
                # A DMA source AP cannot stride across partitions in a
                # non-leading dim, so remap with one plain-slice DMA per
                # (tensor, chunk, head-quarter).
                for tc_ in range(2):
                    for hh_ in range(4):
                        dst = slice(tc_ * 1024 + hh_ * 256,
                                    tc_ * 1024 + hh_ * 256 + 256)
                        srcc = slice(tc_ * 256, tc_ * 256 + 256)
                        srcp = slice(hh_ * 32, hh_ * 32 + 32)
                        nc.sync.dma_start(qt2[:, dst], qt[srcp, srcc])
                        nc.sync.dma_start(kt2[:, dst], kt[srcp, srcc])

                expS = {}
                for kc in range(2):
                    expS[kc] = wp.tile([128, 2048], bf16, tag=f"expS{kc}",
                                       name=f"expS{kc}")
                for hg in range(2):
                    for kc in range(2):
                        sc_ps = psc.tile([128, 1024], f32, tag="sc")
                        for hh in range(4):
                            h = hg * 4 + hh
                            tch, hhh = h // 4, h % 4
                            base = tch * 1024 + hhh * 256
                            nc.tensor.matmul(
                                sc_ps[:, hh * 256:(hh + 1) * 256],
                                kt2[:, base + kc * 128: base + kc * 128 + 128],
                                qt2[:, base: base + 256],
                                start=True, stop=True)
                        # exp(s + mask_k) -> bf16
                        nc.scalar.activation(
                            expS[kc][:, hg * 1024:(hg + 1) * 1024],
                            sc_ps[:, :], AF.Exp,
                            bias=mask_t[:, kc * s_loc + s: kc * s_loc + s + 1])
                # A = expS * exp(pair bias)
                A = wp.tile([128, 4096], bf16, tag="A")
                for kc in range(2):
                    nc.vector.tensor_mul(
                        A[:, kc * 2048:(kc + 1) * 2048],
                        expS[kc][:, :],
                        expb_t[:, kc * 2048:(kc + 1) * 2048])

                # AV: o[q, (h,33)] += A_h^T @ [v_h | 1]
                o_ps = {}
                for qc in range(2):
                    o_ps[qc] = pso.tile([128, 264], f32, tag="o",
                                        name=f"o{qc}")
                for h in range(H):
                    for qc in range(2):
                        for kc in range(2):
                            nc.tensor.matmul(
                                o_ps[qc][:, h * 33: h * 33 + 33],
                                A[:, kc * 2048 + h * 256 + qc * 128:
                                   kc * 2048 + h * 256 + qc * 128 + 128],
                                v_sb[:, kc * 264 + h * 33: kc * 264 + h * 33 + 33],
                                start=(kc == 0), stop=(kc == 1))

                # normalize + gate: og = (gs+1) * (o * (1/Z))
                rz = wp.tile([128, 16], f32, tag="rz")
                t1 = wp.tile([128, 512], f32, tag="t1")
                for qc in range(2):
                    o3 = o_ps[qc].rearrange("p (h e) -> p h e", h=8)
                    nc.vector.reciprocal(
                        rz[:, qc * 8:(qc + 1) * 8], o3[:, :, 32])
                    nc.vector.tensor_mul(
                        t1[:, qc * 256:(qc + 1) * 256].rearrange(
                            "p (h d) -> p h d", h=8),
                        o3[:, :, 0:32],
                        rz[:, qc * 8:(qc + 1) * 8].unsqueeze(2).broadcast_to(
                            (128, 8, 32)))
                og = wp.tile([128, 512], f16, tag="og")
                nc.vector.scalar_tensor_tensor(
                    og[:, :], gs[:, :], 1.0, t1[:, :],
                    op0=ALU.add, op1=ALU.mult)

                # transpose og -> ogT via PE
                tr_ps = pss.tile([128, 512], f16, tag="pss")
                for tcc in range(2):
                    for qc in range(2):
                        nc.tensor.transpose(
                            tr_ps[:, tcc * 256 + qc * 128: tcc * 256 + qc * 128 + 128],
                            og[:, qc * 256 + tcc * 128: qc * 256 + tcc * 128 + 128],
                            id_t[:, :])
                ogt = wp.tile([128, 512], f16, tag="ogt")
                nc.vector.tensor_copy(ogt[:, :], tr_ps[:, :])

                # final projection + bo
                f_ps = pss.tile([128, 512], f32, tag="pss")
                for qc in range(2):
                    for tcc in range(2):
                        nc.tensor.matmul(
                            f_ps[:, qc * 256:(qc + 1) * 256],
                            r(ogt[:, tcc * 256 + qc * 128: tcc * 256 + qc * 128 + 128]),
                            r(wo_t[:, tcc * 256:(tcc + 1) * 256]),
                            start=(tcc == 0), stop=(tcc == 1))
                out_sb = wp.tile([128, 512], f32, tag="out")
                nc.vector.tensor_tensor(
                    out_sb[:, :].rearrange("p (qc c) -> p qc c", qc=2),
                    f_ps[:, :].rearrange("p (qc c) -> p qc c", qc=2),
                    bo_t[:, :].unsqueeze(1).broadcast_to((128, 2, 256)),
                    op=ALU.add)
                nc.sync.dma_start(
                    out_d[s].rearrange("(qc p) c -> p qc c", p=128),
                    out_sb[:, :].rearrange("p (qc c) -> p qc c", qc=2))

    nc.compile()
    return nc


def get_program(s_loc=S_LOC):
    key = (s_loc, os.environ.get('KDTYPE', 'bf16'))
    if key not in _CACHE:
        _CACHE[key] = _build_program(s_loc)
    return _CACHE[key]


def prep_inputs(q_x, kv_x, bias_mask, bias_pair, Wq, Wk, Wv, Wg, bg, Wo, bo,
                s_loc=S_LOC, n_cores=N_CORES):
    """Host-side layout prep. Returns per-core in_maps."""
    bf16 = ml_dtypes.bfloat16

    def wprep(wt):  # (C_in, T_out) -> [p, (cc, t)]
        return np.ascontiguousarray(
            wt.reshape(2, 128, 256).transpose(1, 0, 2).reshape(128, 512)
        ).astype(_mmdt())

    wq_h = wprep(np.asarray(Wq).T)     # lhsT[c, t] = Wq[t, c]
    wk_h = wprep(np.asarray(Wk).T)
    wv_h = wprep(np.asarray(Wv).T)     # rhs[c, t]
    wg_h = wprep(np.asarray(Wg).T)
    wo_h = wprep(np.asarray(Wo).T * 0.5)  # rhs[t, c] = Wo[c, t]; 0.5 = sigmoid fold
    bg_h = np.asarray(bg, _mmdt()).reshape(1, 256)
    bo_h = np.ascontiguousarray(np.broadcast_to(
        np.asarray(bo, np.float32), (128, 256)))
    id_h = np.eye(128, dtype=_mmdt())

    eb = np.exp(np.asarray(bias_pair[0, 0], np.float64)).astype(np.float32)
    ebT = eb.transpose(0, 2, 1)  # (H, K, Q)
    expb_h = np.ascontiguousarray(
        ebT.reshape(H, 2, 128, Q).transpose(2, 1, 0, 3).reshape(128, 2 * H * Q)
    ).astype(bf16)

    xq_all = np.ascontiguousarray(
        np.asarray(q_x[0], _mmdt()).transpose(0, 2, 1))   # (S, C, Q)
    xkv_all = np.ascontiguousarray(
        np.asarray(kv_x[0], _mmdt()).transpose(0, 2, 1))
    mask_all = np.asarray(bias_mask[0, :, 0, 0, :], np.float32)  # (S, K)

    in_maps = []
    for core in range(n_cores):
        lo = core * s_loc
        m = mask_all[lo:lo + s_loc]  # (s_loc, K)
        mask_h = np.ascontiguousarray(
            m.T.reshape(2, 128, s_loc).transpose(1, 0, 2).reshape(128, 2 * s_loc))
        in_maps.append({
            "xq": xq_all[lo:lo + s_loc],
            "xkv": xkv_all[lo:lo + s_loc],
            "maskt": mask_h,
            "expb": expb_h,
            "wq": wq_h, "wk": wk_h, "wv": wv_h, "wg": wg_h, "wo": wo_h,
            "bg": bg_h, "bo": bo_h, "ident": id_h,
            "ones": np.ones((1, 128), _mmdt()),
        })
    return in_maps


def kernel(q_x, kv_x, bias_mask, bias_pair, Wq, Wk, Wv, Wg, bg, Wo, bo):
    from concourse import bass_utils

    nc = get_program()
    in_maps = prep_inputs(q_x, kv_x, bias_mask, bias_pair,
                          Wq, Wk, Wv, Wg, bg, Wo, bo)
    res = bass_utils.run_bass_kernel_spmd(
        nc, in_maps, core_ids=list(range(N_CORES)))
    out = np.concatenate([res.results[i]["out"] for i in range(N_CORES)], axis=0)
    return out.reshape(B, S, Q, C).astype(np.float32)


# revision 23
# speedup vs baseline: 1.1564x; 1.1564x over previous
"""Trainium2 Bass kernel for the sparse_attention nn.Module problem.

Strategy: data-parallel over the MSA-row dim S (S=128 -> 16 rows per core,
8 cores). All projection weights + pair bias replicated; mask bias and
activations sharded with S. No collectives.

Per-core dataflow (all layouts chosen so no on-device input transposes are
needed; host pre-transposes x to [s, c, q]):
  qT/kT = W @ x^T            (fp32r matmuls, PSUM fp32, DVE evict to SBUF)
  v     = kv_x @ Wv^T        (natural [k, t] layout, evicted to bf16 with a
                              ones column per head for the softmax sum)
  g     = q_x @ Wg^T + bg    (bg added via a rank-1 K=1 matmul; sigmoid done
                              as 0.5*(1+tanh(x/2)) with the 0.5 folded into Wo)
  sT_h  = kT_h^T @ qT_h      (scores transposed: [k, q], per head)
  expS  = exp(sT + mask)     (ACT, mask is per-partition bias; no max-sub --
                              |logits| <= ~70 so fp32/bf16 exp is safe)
  A     = expS * exp(pairT)  (exp(bias_pair) precomputed on host, bf16)
  o,Z   = A^T @ [v | 1]      (AV matmul in natural layout, N=33 per head;
                              col 32 accumulates Z = sum_k A)
  og    = (gs+1) * (o * (1/Z))
  ogT   = PE transpose(og)
  out   = ogT^T @ (0.5*Wo)^T + bo
"""

import os
import numpy as np
import ml_dtypes

def _mmdt():
    return (ml_dtypes.bfloat16 if os.environ.get('KDTYPE', 'fp16') == 'bf16'
            else np.float16)

B, S, Q, C = 1, 128, 256, 256
H, DH = 8, 32
TOT = H * DH
N_CORES = 8
S_LOC = S // N_CORES  # 16

_CACHE = {}


def _build_program(s_loc):
    import concourse.bacc as bacc
    import concourse.mybir as mybir
    from concourse import tile

    dt = mybir.dt
    f32, bf16 = dt.float32, dt.bfloat16
    f16 = bf16 if os.environ.get('KDTYPE', 'fp16') == 'bf16' else dt.float16
    AF = mybir.ActivationFunctionType
    ALU = mybir.AluOpType

    nc = bacc.Bacc("TRN2", target_bir_lowering=False, debug=False,
                   num_devices=N_CORES)

    x_d = nc.dram_tensor("x", [s_loc, 2 * C, Q], f16, kind="ExternalInput").ap()
    mask_d = nc.dram_tensor("maskt", [128, 2 * s_loc], f32, kind="ExternalInput").ap()
    expb_d = nc.dram_tensor("expb", [128, 2 * H * Q], bf16, kind="ExternalInput").ap()
    wq_d = nc.dram_tensor("wq", [128, 512], f16, kind="ExternalInput").ap()
    wk_d = nc.dram_tensor("wk", [128, 512], f16, kind="ExternalInput").ap()
    wv_d = nc.dram_tensor("wv", [128, 512], f16, kind="ExternalInput").ap()
    wg_d = nc.dram_tensor("wg", [128, 512], f16, kind="ExternalInput").ap()
    wo_d = nc.dram_tensor("wo", [128, 512], f16, kind="ExternalInput").ap()
    bg_d = nc.dram_tensor("bg", [1, 256], f16, kind="ExternalInput").ap()
    bo_d = nc.dram_tensor("bo", [128, 256], f32, kind="ExternalInput").ap()
    id_d = nc.dram_tensor("ident", [128, 128], f16, kind="ExternalInput").ap()
    ones_d = nc.dram_tensor("ones", [1, 128], f16, kind="ExternalInput").ap()
    out_d = nc.dram_tensor("out", [s_loc, Q, C], f32, kind="ExternalOutput").ap()

    def r(ap):
        return ap

    with tile.TileContext(nc) as tc:
        with (
            tc.tile_pool(name="const", bufs=1) as cp,
            tc.tile_pool(name="work", bufs=2) as wp,
            tc.tile_pool(name="ps_small", bufs=2, space="PSUM") as pss,
            tc.tile_pool(name="ps_sc", bufs=2, space="PSUM") as psc,
            tc.tile_pool(name="ps_o", bufs=2, space="PSUM") as pso,
        ):
            # ---- resident constants ----
            wq_t = cp.tile([128, 512], f16, tag="wq")
            wk_t = cp.tile([128, 512], f16, tag="wk")
            wv_t = cp.tile([128, 512], f16, tag="wv")
            wg_t = cp.tile([128, 512], f16, tag="wg")
            wo_t = cp.tile([128, 512], f16, tag="wo")
            expb_t = cp.tile([128, 2 * H * Q], bf16, tag="expb")
            mask_t = cp.tile([128, 2 * s_loc], f32, tag="mask")
            bg_t = cp.tile([1, 256], f16, tag="bg")
            bo_t = cp.tile([128, 256], f32, tag="bo")
            id_t = cp.tile([128, 128], f16, tag="ident")
            ones_t = cp.tile([1, 128], f16, tag="ones")

            nc.sync.dma_start(wq_t[:, :], wq_d[:, :])
            nc.sync.dma_start(wk_t[:, :], wk_d[:, :])
            nc.sync.dma_start(wv_t[:, :], wv_d[:, :])
            nc.sync.dma_start(wg_t[:, :], wg_d[:, :])
            nc.sync.dma_start(wo_t[:, :], wo_d[:, :])
            nc.sync.dma_start(expb_t[:, :], expb_d[:, :])
            nc.sync.dma_start(mask_t[:, :], mask_d[:, :])
            nc.sync.dma_start(bg_t[:, :], bg_d[:, :])
            nc.sync.dma_start(bo_t[:, :], bo_d[:, :])
            nc.sync.dma_start(id_t[:, :], id_d[:, :])
            nc.sync.dma_start(ones_t[:, :], ones_d[:, :])

            for s in range(s_loc):
                # ---- load x^T shards (xq | xkv in one tensor) ----
                xx = wp.tile([128, 1024], f16, tag="xx")
                nc.sync.dma_start(
                    xx[:, :].rearrange("p (cc q) -> p cc q", cc=4),
                    x_d[s].rearrange("(cc p) q -> p cc q", p=128))
                xq = xx[:, 0:512]
                xkv = xx[:, 512:1024]

                # ---- projections (fp32r) ----
                # qT[t, q] += WqT[c, t]^T @ xqT[c, q]
                qt_ps = pss.tile([128, 512], f32, tag="pss")
                for tcc in range(2):
                    for cc in range(2):
                        nc.tensor.matmul(
                            qt_ps[:, tcc * 256:(tcc + 1) * 256],
                            r(wq_t[:, cc * 256 + tcc * 128: cc * 256 + tcc * 128 + 128]),
                            r(xq[:, cc * 256:(cc + 1) * 256]),
                            start=(cc == 0), stop=(cc == 1))
                qt = wp.tile([128, 512], f16, tag="qt")
                ev_q = nc.vector.tensor_copy(qt[:, :], qt_ps[:, :])

                kt_ps = pss.tile([128, 512], f32, tag="pss")
                for tcc in range(2):
                    for cc in range(2):
                        nc.tensor.matmul(
                            kt_ps[:, tcc * 256:(tcc + 1) * 256],
                            r(wk_t[:, cc * 256 + tcc * 128: cc * 256 + tcc * 128 + 128]),
                            r(xkv[:, cc * 256:(cc + 1) * 256]),
                            start=(cc == 0), stop=(cc == 1))
                kt = wp.tile([128, 512], f16, tag="kt")
                ev_k = nc.vector.tensor_copy(kt[:, :], kt_ps[:, :])

                # v natural [k, t]
                v_ps = pss.tile([128, 512], f32, tag="pss")
                for kc in range(2):
                    for cc in range(2):
                        nc.tensor.matmul(
                            v_ps[:, kc * 256:(kc + 1) * 256],
                            r(xkv[:, cc * 256 + kc * 128: cc * 256 + kc * 128 + 128]),
                            r(wv_t[:, cc * 256:(cc + 1) * 256]),
                            start=(cc == 0), stop=(cc == 1))
                # v_aug bf16 [k, (kc, h, 33)]; col 32 of each head = 1.0
                v_sb = wp.tile([128, 528], bf16, tag="v")
                v4 = v_sb.rearrange("p (kc h e) -> p kc h e", kc=2, h=8)
                nc.gpsimd.memset(v4[:, :, :, 32], 1.0)
                for kc in range(2):
                    nc.vector.tensor_copy(
                        v4[:, kc, :, 0:32],
                        v_ps[:, kc * 256:(kc + 1) * 256].rearrange(
                            "p (h d) -> p h d", h=8))

                # g natural [q, t] with bg via rank-1 matmul
                g_ps = pss.tile([128, 512], f32, tag="pss")
                for qc in range(2):
                    for cc in range(2):
                        nc.tensor.matmul(
                            g_ps[:, qc * 256:(qc + 1) * 256],
                            r(xq[:, cc * 256 + qc * 128: cc * 256 + qc * 128 + 128]),
                            r(wg_t[:, cc * 256:(cc + 1) * 256]),
                            start=(cc == 0), stop=False)
                    nc.tensor.matmul(
                        g_ps[:, qc * 256:(qc + 1) * 256],
                        r(ones_t[:, :]), r(bg_t[:, :]),
                        start=False, stop=True)
                # gs = tanh(g/2); sigmoid = 0.5*(gs+1), 0.5 folded into Wo
                gs = wp.tile([128, 512], f32, tag="gs")
                nc.scalar.activation(gs[:, :], g_ps[:, :], AF.Tanh, scale=0.5)

                # ---- attention ----
                # Mixing PE tile positions crashes this runtime, so every
                # matmul must sit at partition base 0: DMA-remap qt/kt from
                # [(hh,d), (tc,q)] to head-flat [d, (tc,hh,q)].
                qt2 = wp.tile([32, 2048], f16, tag="qt2")
                kt2 = wp.tile([32, 2048], f16, tag="kt2")
                # A DMA source AP cannot stride across partitions in a
                # non-leading dim, so remap per head-quarter (both t-chunks
                # in one 3D-AP DMA). Triggers go on otherwise-idle engines
                # to keep the sync queue off the critical path.
                for hh_ in range(4):
                    srcp = slice(hh_ * 32, hh_ * 32 + 32)
                    nc.gpsimd.dma_start(
                        qt2.rearrange("d (tc x) -> d tc x", tc=2)[
                            :, :, hh_ * 256: hh_ * 256 + 256],
                        qt[srcp, :].rearrange("d (tc q) -> d tc q", tc=2))
                    nc.scalar.dma_start(
                        kt2.rearrange("d (tc x) -> d tc x", tc=2)[
                            :, :, hh_ * 256: hh_ * 256 + 256],
                        kt[srcp, :].rearrange("d (tc q) -> d tc q", tc=2))

                expS = {}
                for kc in range(2):
                    expS[kc] = wp.tile([128, 2048], bf16, tag=f"expS{kc}",
                                       name=f"expS{kc}")
                for hg in range(2):
                    for kc in range(2):
                        sc_ps = psc.tile([128, 1024], f32, tag="sc")
                        for hh in range(4):
                            h = hg * 4 + hh
                            tch, hhh = h // 4, h % 4
                            base = tch * 1024 + hhh * 256
                            nc.tensor.matmul(
                                sc_ps[:, hh * 256:(hh + 1) * 256],
                                kt2[:, base + kc * 128: base + kc * 128 + 128],
                                qt2[:, base: base + 256],
                                start=True, stop=True)
                        # exp(s + mask_k) -> bf16
                        nc.scalar.activation(
                            expS[kc][:, hg * 1024:(hg + 1) * 1024],
                            sc_ps[:, :], AF.Exp,
                            bias=mask_t[:, kc * s_loc + s: kc * s_loc + s + 1])
                # A = expS * exp(pair bias)
                A = wp.tile([128, 4096], bf16, tag="A")
                for kc in range(2):
                    nc.vector.tensor_mul(
                        A[:, kc * 2048:(kc + 1) * 2048],
                        expS[kc][:, :],
                        expb_t[:, kc * 2048:(kc + 1) * 2048])

                # AV: o[q, (h,33)] += A_h^T @ [v_h | 1]
                o_ps = {}
                for qc in range(2):
                    o_ps[qc] = pso.tile([128, 264], f32, tag="o",
                                        name=f"o{qc}")
                for h in range(H):
                    for qc in range(2):
                        for kc in range(2):
                            nc.tensor.matmul(
                                o_ps[qc][:, h * 33: h * 33 + 33],
                                A[:, kc * 2048 + h * 256 + qc * 128:
                                   kc * 2048 + h * 256 + qc * 128 + 128],
                                v_sb[:, kc * 264 + h * 33: kc * 264 + h * 33 + 33],
                                start=(kc == 0), stop=(kc == 1))

                # normalize + gate: og = (gs+1) * (o * (1/Z))
                rz = wp.tile([128, 16], f32, tag="rz")
                t1 = wp.tile([128, 512], f32, tag="t1")
                for qc in range(2):
                    o3 = o_ps[qc].rearrange("p (h e) -> p h e", h=8)
                    nc.vector.reciprocal(
                        rz[:, qc * 8:(qc + 1) * 8], o3[:, :, 32])
                    nc.vector.tensor_mul(
                        t1[:, qc * 256:(qc + 1) * 256].rearrange(
                            "p (h d) -> p h d", h=8),
                        o3[:, :, 0:32],
                        rz[:, qc * 8:(qc + 1) * 8].unsqueeze(2).broadcast_to(
                            (128, 8, 32)))
                og = wp.tile([128, 512], f16, tag="og")
                nc.vector.scalar_tensor_tensor(
                    og[:, :], gs[:, :], 1.0, t1[:, :],
                    op0=ALU.add, op1=ALU.mult)

                # transpose og -> ogT via PE
                tr_ps = pss.tile([128, 512], f16, tag="pss")
                for tcc in range(2):
                    for qc in range(2):
                        nc.tensor.transpose(
                            tr_ps[:, tcc * 256 + qc * 128: tcc * 256 + qc * 128 + 128],
                            og[:, qc * 256 + tcc * 128: qc * 256 + tcc * 128 + 128],
                            id_t[:, :])
                ogt = wp.tile([128, 512], f16, tag="ogt")
                nc.vector.tensor_copy(ogt[:, :], tr_ps[:, :])

                # final projection + bo
                f_ps = pss.tile([128, 512], f32, tag="pss")
                for qc in range(2):
                    for tcc in range(2):
                        nc.tensor.matmul(
                            f_ps[:, qc * 256:(qc + 1) * 256],
                            r(ogt[:, tcc * 256 + qc * 128: tcc * 256 + qc * 128 + 128]),
                            r(wo_t[:, tcc * 256:(tcc + 1) * 256]),
                            start=(tcc == 0), stop=(tcc == 1))
                out_sb = wp.tile([128, 512], f32, tag="out")
                nc.vector.tensor_tensor(
                    out_sb[:, :].rearrange("p (qc c) -> p qc c", qc=2),
                    f_ps[:, :].rearrange("p (qc c) -> p qc c", qc=2),
                    bo_t[:, :].unsqueeze(1).broadcast_to((128, 2, 256)),
                    op=ALU.add)
                nc.sync.dma_start(
                    out_d[s].rearrange("(qc p) c -> p qc c", p=128),
                    out_sb[:, :].rearrange("p (qc c) -> p qc c", qc=2))

    nc.compile()
    return nc


def get_program(s_loc=S_LOC):
    key = (s_loc, os.environ.get('KDTYPE', 'bf16'))
    if key not in _CACHE:
        _CACHE[key] = _build_program(s_loc)
    return _CACHE[key]


def prep_inputs(q_x, kv_x, bias_mask, bias_pair, Wq, Wk, Wv, Wg, bg, Wo, bo,
                s_loc=S_LOC, n_cores=N_CORES):
    """Host-side layout prep. Returns per-core in_maps."""
    bf16 = ml_dtypes.bfloat16

    def wprep(wt):  # (C_in, T_out) -> [p, (cc, t)]
        return np.ascontiguousarray(
            wt.reshape(2, 128, 256).transpose(1, 0, 2).reshape(128, 512)
        ).astype(_mmdt())

    wq_h = wprep(np.asarray(Wq).T)     # lhsT[c, t] = Wq[t, c]
    wk_h = wprep(np.asarray(Wk).T)
    wv_h = wprep(np.asarray(Wv).T)     # rhs[c, t]
    wg_h = wprep(np.asarray(Wg).T)
    wo_h = wprep(np.asarray(Wo).T * 0.5)  # rhs[t, c] = Wo[c, t]; 0.5 = sigmoid fold
    bg_h = np.asarray(bg, _mmdt()).reshape(1, 256)
    bo_h = np.ascontiguousarray(np.broadcast_to(
        np.asarray(bo, np.float32), (128, 256)))
    id_h = np.eye(128, dtype=_mmdt())

    eb = np.exp(np.asarray(bias_pair[0, 0], np.float64)).astype(np.float32)
    ebT = eb.transpose(0, 2, 1)  # (H, K, Q)
    expb_h = np.ascontiguousarray(
        ebT.reshape(H, 2, 128, Q).transpose(2, 1, 0, 3).reshape(128, 2 * H * Q)
    ).astype(bf16)

    x_all = np.concatenate([
        np.asarray(q_x[0], _mmdt()).transpose(0, 2, 1),
        np.asarray(kv_x[0], _mmdt()).transpose(0, 2, 1)], axis=1)
    x_all = np.ascontiguousarray(x_all)   # (S, 2C, Q): xq | xkv
    mask_all = np.asarray(bias_mask[0, :, 0, 0, :], np.float32)  # (S, K)

    in_maps = []
    for core in range(n_cores):
        lo = core * s_loc
        m = mask_all[lo:lo + s_loc]  # (s_loc, K)
        mask_h = np.ascontiguousarray(
            m.T.reshape(2, 128, s_loc).transpose(1, 0, 2).reshape(128, 2 * s_loc))
        in_maps.append({
            "x": x_all[lo:lo + s_loc],
            "maskt": mask_h,
            "expb": expb_h,
            "wq": wq_h, "wk": wk_h, "wv": wv_h, "wg": wg_h, "wo": wo_h,
            "bg": bg_h, "bo": bo_h, "ident": id_h,
            "ones": np.ones((1, 128), _mmdt()),
        })
    return in_maps


def kernel(q_x, kv_x, bias_mask, bias_pair, Wq, Wk, Wv, Wg, bg, Wo, bo):
    from concourse import bass_utils

    nc = get_program()
    in_maps = prep_inputs(q_x, kv_x, bias_mask, bias_pair,
                          Wq, Wk, Wv, Wg, bg, Wo, bo)
    res = bass_utils.run_bass_kernel_spmd(
        nc, in_maps, core_ids=list(range(N_CORES)))
    out = np.concatenate([res.results[i]["out"] for i in range(N_CORES)], axis=0)
    return out.reshape(B, S, Q, C).astype(np.float32)
